# revision 22
# baseline (speedup 1.0000x reference)
"""Trainium2 Bass kernel for AttentionSR (spatial-reduction attention).

Reference computation (per batch b):
  q = x @ Wq.T                                   [4096, 512] -> heads [8, 4096, 64]
  x_ = conv2x2_stride2(x as NCHW image, Wsr) + bsr   -> [1024, 512]
  x_ = layernorm(x_, g, b)
  k, v = split(x_ @ Wkv.T)                       [8, 1024, 64] each
  out = softmax(q k^T / 8) v                     -> [4096, 512]
  y = out @ Wp.T + bp

Sharding (8 cores): core = 2*batch + query_half. Each core owns one batch's
conv/LN/KV (duplicated across the pair) and 2048 of its 4096 query rows.
No collectives.

On-device layout is channel-major throughout (host passes x transposed), so
the kernel needs no PE transposes:
  conv/q/k projections keep channels on partitions; v is produced token-major
  by swapping the stationary matmul operand; attention scores are computed
  transposed [keys, q]; the softmax denominator comes from a ones column
  appended to v in the attn@v stationary operand; the final division is an
  approx-reciprocal + gpsimd partition-broadcast + multiply on [64, q] tiles.
Matmul paths run in bf16 (weight loads overlap via FWL, N=1024 moving
operands); LN statistics run in float32r for accuracy; psums are fp32.
LayerNorm is folded into the KV projection: x_scaled = x_raw * rstd plus two
extension rows (s2 = -mu*rstd, ones) against host-extended weights
[ (W*g).T ; W@g ; W@b ].
Attention processes head pairs: the two K=64 score matmuls row-pack into
array rows 0-63 / 64-127 (concurrent), and both heads' attn@v accumulate
into one [65, 2048] psum.
"""

import numpy as np
import ml_dtypes
from contextlib import ExitStack

import concourse.bass as bass
import concourse.bacc as bacc
import concourse.tile as tile
from concourse import mybir
from concourse.bass_utils import run_bass_kernel_spmd

BF = ml_dtypes.bfloat16
F32 = mybir.dt.float32
F32R = mybir.dt.float32r
BF16 = mybir.dt.bfloat16
AF = mybir.ActivationFunctionType
ALU = mybir.AluOpType

C = 512          # model dim
NHEAD = 8
DH = 64          # head dim
HS = WS = 64     # image height/width
NTOK = HS * WS   # 4096 tokens per batch
NQ = 2048        # query rows per core
NKV = 1024       # reduced tokens (keys)
B = 4
SCALE = DH ** -0.5
EPS = 1e-5


def _emit(nc, tc, ctx, io, dbg=None):
    (xq, xo, w2, wq, wkg, wkg2, wvg, wvg2, wp, bsr_t, bp_t,
     ones_row, ones_col, ones_c1, yt) = io

    pp = ctx.enter_context(tc.tile_pool(name="pp", bufs=2, space="PSUM"))
    pav = ctx.enter_context(tc.tile_pool(name="pav", bufs=2, space="PSUM"))
    persist = ctx.enter_context(tc.tile_pool(name="persist", bufs=1))
    small = ctx.enter_context(tc.tile_pool(name="small", bufs=1))

    # ---- persistent sbuf tensors ----
    qT = [persist.tile([128, NQ], BF16, tag=f"qT{i}", name=f"qT{i}") for i in range(4)]
    kT0 = [persist.tile([128, 512], BF16, tag=f"kT0{i}", name=f"kT0{i}") for i in range(4)]
    kT1 = [persist.tile([128, 512], BF16, tag=f"kT1{i}", name=f"kT1{i}") for i in range(4)]
    v_sb = [persist.tile([128, NHEAD, DH + 1], BF16, tag=f"v{i}", name=f"v{i}")
            for i in range(8)]
    vout = [[persist.tile([128, 1024], BF16, tag=f"vout{i}_{h}", name=f"vout{i}_{h}")
             for h in range(2)] for i in range(4)]
    x_raw = [persist.tile([128, NKV], F32R, tag=f"xraw{i}", name=f"xraw{i}")
             for i in range(4)]
    wp_sb = persist.tile([128, 4, 512], BF16, tag="wp")

    bsr_sb = small.tile([128, 4], F32)
    nc.sync.dma_start(out=bsr_sb[:], in_=bsr_t)
    bp_sb = small.tile([128, 4], F32)
    nc.sync.dma_start(out=bp_sb[:], in_=bp_t)
    ones_c = small.tile([128, 1], F32R)
    nc.sync.dma_start(out=ones_c[:], in_=ones_c1)
    # LN row tensors ([1, N] tiles, base partition 0; values overwritten in place)
    sum_row = small.tile([1, NKV], F32)    # sum -> m
    sq_row = small.tile([1, NKV], F32)     # sumsq -> var -> std -> rstd
    msq_row = small.tile([1, NKV], F32)    # m^2 / newton scratch
    y_row = small.tile([1, NKV], F32)      # rsqrt newton iterate
    rstd_bc = small.tile([128, NKV], F32)
    xs_ext2 = small.tile([2, NKV], BF16)   # row0 = -mu*rstd, row1 = ones (DMA)
    nc.sync.dma_start(out=xs_ext2[1:2, :], in_=ones_row)

    # ========== Phase A+B: per-half pipeline: conv -> LN -> KV; then q ==========
    xs_ln = [persist.tile([128, NKV], BF16, tag=f"xsln{i}", name=f"xsln{i}")
             for i in range(4)]
    with tc.tile_pool(name="pA", bufs=1) as pA, \
         tc.tile_pool(name="pB", bufs=4) as pB, \
         tc.tile_pool(name="xstream", bufs=2) as pX:
        xh0 = pX.tile([128, 4, NQ], BF16, tag="xh", name="xh")
        nc.sync.dma_start(out=xh0[:], in_=xq.rearrange("(ct p) t -> p ct t", p=128))
        w2_sb = [pA.tile([128, 4, 512], BF16, tag=f"w2_{i}", name=f"w2_{i}")
                 for i in range(4)]
        w2v = w2.rearrange("(dd ct p) o -> dd p ct o", dd=4, p=128)
        for i in range(4):
            nc.sync.dma_start(out=w2_sb[i][:], in_=w2v[i])
        wq_sb = pA.tile([128, 4, 512], BF16)
        nc.sync.dma_start(out=wq_sb[:], in_=wq.rearrange("(ct p) o -> p ct o", p=128))
        wkg_sb = persist.tile([128, 4, 512], BF16, tag="wkg", name="wkg_sb")
        nc.sync.dma_start(out=wkg_sb[:], in_=wkg.rearrange("(ct p) o -> p ct o", p=128))
        wkg2_sb = persist.tile([2, 512], BF16, tag="wkg2", name="wkg2_sb")
        nc.sync.dma_start(out=wkg2_sb[:], in_=wkg2)
        wvg_sb = persist.tile([128, 4, 512], BF16, tag="wvg", name="wvg_sb")
        nc.sync.dma_start(out=wvg_sb[:], in_=wvg.rearrange("(ct p) o -> p ct o", p=128))
        wvg2_sb = persist.tile([2, 512], BF16, tag="wvg2", name="wvg2_sb")
        nc.sync.dma_start(out=wvg2_sb[:], in_=wvg2)

        inv_c = 1.0 / C
        xh_q = None

        def conv_half(half, xh):
            hsl = slice(half * 512, (half + 1) * 512)
            for ot in range(4):
                ps = pp.tile([128, 1024], F32, tag="ps", name="ps_conv")
                psv = ps[:, 0:512].rearrange("p (a b) -> p a b", a=16)
                for kk in range(16):
                    di, dj, ct = kk // 8, (kk // 4) % 2, kk % 4
                    rhs = bass.AP(
                        tensor=xh[:].tensor,
                        offset=xh[:].offset + ct * NQ + di * WS + dj,
                        ap=[xh[:].ap[0], [2 * WS, 16], [2, 32]],
                    )
                    nc.tensor.matmul(
                        psv, lhsT=w2_sb[kk // 4][:, kk % 4, ot * 128:(ot + 1) * 128],
                        rhs=rhs, start=(kk == 0), stop=(kk == 15),
                    )
                nc.vector.tensor_scalar_add(
                    x_raw[ot][:, hsl], ps[:, 0:512], bsr_sb[:, ot:ot + 1],
                )

        def stats_half(half):
            hsl = slice(half * 512, (half + 1) * 512)
            xsq = []
            for ct in range(4):
                t = pB.tile([128, 512], F32R, tag="xsq", name="xsq")
                nc.vector.tensor_mul(t[:], x_raw[ct][:, hsl].bitcast(F32),
                                     x_raw[ct][:, hsl].bitcast(F32))
                xsq.append(t)
            ps = pp.tile([128, 1024], F32, tag="ps", name="ps_sum")
            for ct in range(4):
                nc.tensor.matmul(ps[0:1, 0:512], lhsT=ones_c[:], rhs=x_raw[ct][:, hsl],
                                 start=(ct == 0), stop=(ct == 3))
            for ct in range(4):
                nc.tensor.matmul(ps[0:1, 512:1024], lhsT=ones_c[:], rhs=xsq[ct][:],
                                 start=(ct == 0), stop=(ct == 3))
            nc.vector.tensor_copy(sum_row[0:1, hsl], ps[0:1, 0:512])
            nc.vector.tensor_copy(sq_row[0:1, hsl], ps[0:1, 512:1024])

        def ln_rows_half(half):
            hsl = slice(half * 512, (half + 1) * 512)
            sm, sq, mq = sum_row[0:1, hsl], sq_row[0:1, hsl], msq_row[0:1, hsl]
            yy = y_row[0:1, hsl]
            nc.vector.tensor_scalar_mul(sm, sm, inv_c)              # m
            nc.vector.tensor_mul(mq, sm, sm)                        # m^2
            nc.vector.scalar_tensor_tensor(sq, sq, inv_c, mq,
                                           op0=ALU.mult, op1=ALU.subtract)   # var
            nc.vector.tensor_scalar_add(sq, sq, EPS)                # var + eps
            # rstd = rsqrt(var+eps): seed 1/v, 3 Newton steps
            nc.vector.reciprocal_approx_fast(out=yy, in_=sq)
            for _ in range(3):
                nc.vector.tensor_mul(mq, yy, yy)
                nc.vector.scalar_tensor_tensor(mq, sq, -0.5, mq,
                                               op0=ALU.mult, op1=ALU.mult)
                nc.vector.scalar_tensor_tensor(yy, mq, 1.5, yy,
                                               op0=ALU.add, op1=ALU.mult)
            nc.vector.scalar_tensor_tensor(xs_ext2[0:1, hsl], sm, -1.0, yy,
                                           op0=ALU.mult, op1=ALU.mult)       # s2
            nc.gpsimd.partition_broadcast(rstd_bc[:, hsl], yy)
            for ct in range(4):
                nc.vector.tensor_mul(xs_ln[ct][:, hsl],
                                     x_raw[ct][:, hsl].bitcast(F32),
                                     rstd_bc[:, hsl])

        def kv_half(half):
            hsl = slice(half * 512, (half + 1) * 512)
            kTh = kT0 if half == 0 else kT1
            for ot in range(4):
                ps = pp.tile([128, 1024], F32, tag="ps", name="ps_k")
                for ct in range(4):
                    nc.tensor.matmul(ps[:, 0:512],
                                     lhsT=wkg_sb[:, ct, ot * 128:(ot + 1) * 128],
                                     rhs=xs_ln[ct][:, hsl],
                                     start=(ct == 0), stop=False)
                nc.tensor.matmul(ps[:, 0:512],
                                 lhsT=wkg2_sb[:, ot * 128:(ot + 1) * 128],
                                 rhs=xs_ext2[:, hsl], start=False, stop=True)
                nc.vector.tensor_copy(kTh[ot][:], ps[:, 0:512])
            for tt in range(half * 4, half * 4 + 4):
                sl = slice(tt * 128, (tt + 1) * 128)
                ps = pp.tile([128, 1024], F32, tag="ps", name="ps_v")
                for ct in range(4):
                    nc.tensor.matmul(ps[:, 0:512], lhsT=xs_ln[ct][:, sl],
                                     rhs=wvg_sb[:, ct, :], start=(ct == 0), stop=False)
                nc.tensor.matmul(ps[:, 0:512], lhsT=xs_ext2[:, sl], rhs=wvg2_sb[:],
                                 start=False, stop=True)
                nc.vector.tensor_copy(
                    v_sb[tt][:, :, 0:DH],
                    ps[:, 0:512].rearrange("p (h d) -> p h d", h=NHEAD),
                )

        xh1 = pX.tile([128, 4, NQ], BF16, tag="xh", name="xh1")
        nc.sync.dma_start(out=xh1[:], in_=xo.rearrange("(ct p) t -> p ct t", p=128))
        for i in range(8):
            nc.sync.dma_start(out=v_sb[i][:, :, DH:DH + 1], in_=ones_col)
        conv_half(0, xh0)
        stats_half(0)
        ln_rows_half(0)          # DVE chain overlaps the q/conv1 matmuls below
        for ot in range(4):      # q projection early (unblocks attention sooner)
            for qc in range(2):
                ps = pp.tile([128, 1024], F32, tag="ps", name="ps_q")
                for ct in range(4):
                    for nn in range(2):
                        nc.tensor.matmul(
                            ps[:, nn * 512:(nn + 1) * 512],
                            lhsT=wq_sb[:, ct, ot * 128:(ot + 1) * 128],
                            rhs=xh0[:, ct, qc * 1024 + nn * 512:
                                    qc * 1024 + nn * 512 + 512],
                            start=(ct == 0), stop=(ct == 3),
                        )
                nc.vector.tensor_copy(qT[ot][:, qc * 1024:(qc + 1) * 1024], ps[:])
        conv_half(1, xh1)
        kv_half(0)
        stats_half(1)
        ln_rows_half(1)
        _emit._kv1 = kv_half

    nc.sync.dma_start(out=wp_sb[:], in_=wp.rearrange("(ct p) o -> p ct o", p=128))
    # ================= Phase C: attention (head pairs, 512-col q chunks) ======
    with tc.tile_pool(name="pexp", bufs=8) as pexp, \
         tc.tile_pool(name="psig", bufs=2) as psig, \
         tc.tile_pool(name="py", bufs=3) as py:

        def proj_chunk(qh, ot, nns=(0, 1)):
            for nn in nns:
                ps = pp.tile([128, 1024], F32, tag="ps", name="ps_proj")
                for ct in range(4):
                    nc.tensor.matmul(
                        ps[:, 0:512],
                        lhsT=wp_sb[:, ct, ot * 128:(ot + 1) * 128],
                        rhs=vout[ct][qh][:, nn * 512:(nn + 1) * 512],
                        start=(ct == 0), stop=(ct == 3))
                yt_t = py.tile([128, 1024], F32, tag="y", name="yt_t")
                nc.vector.tensor_scalar_add(yt_t[:, 0:512], ps[:, 0:512],
                                            bp_sb[:, ot:ot + 1])
                nc.sync.dma_start(
                    out=yt[ot * 128:(ot + 1) * 128,
                           qh * 1024 + nn * 512:qh * 1024 + nn * 512 + 512],
                    in_=yt_t[:, 0:512])

        def attn_kts(hp, qc, av, kts):
            qsl = slice(qc * 512, (qc + 1) * 512)
            for kt in kts:
                kTh = kT0 if kt < 4 else kT1
                ksl = slice((kt % 4) * 128, (kt % 4) * 128 + 128)
                sc = pp.tile([128, 1024], F32, tag="ps", name="sc")
                # both heads' score matmuls adjacent -> concurrent row groups
                for sub in range(2):
                    rr = sub * 64
                    nc.tensor.matmul(
                        sc[:, sub * 512:(sub + 1) * 512],
                        lhsT=kTh[hp][rr:rr + 64, ksl],
                        rhs=qT[hp][rr:rr + 64, qsl],
                        start=True, stop=True,
                    )
                ex = pexp.tile([128, 1024], BF16, tag="exp", name="ex")
                nc.scalar.activation(ex[:], sc[:], AF.Exp, scale=SCALE)
                for sub in range(2):
                    nc.tensor.matmul(
                        av[:, sub * 512:(sub + 1) * 512],
                        lhsT=v_sb[kt][:, 2 * hp + sub, :],
                        rhs=ex[:, sub * 512:(sub + 1) * 512],
                        start=(kt == 0), stop=(kt == 7),
                    )

        def attn_evac(hp, qc, av):
            qh, qr = qc // 2, (qc % 2) * 512
            sig = psig.tile([1, 1024], F32, tag="sig", name="sig")
            nc.vector.tensor_copy(sig[:], av[64:65, :])
            rbc = psig.tile([64, 1024], F32, tag="rbc", name="rbc")
            nc.vector.reciprocal_approx_fast(out=rbc[0:1, :], in_=sig[:])
            nc.gpsimd.partition_broadcast(rbc[:], rbc[0:1, :])
            nc.vector.tensor_mul(vout[hp][qh][0:64, qr:qr + 512],
                                 av[0:64, 0:512], rbc[:, 0:512])
            nc.vector.tensor_mul(vout[hp][qh][64:128, qr:qr + 512],
                                 av[0:64, 512:1024], rbc[:, 512:1024])

        # qc=0: first two passes split around the deferred second-half KV so
        # their kt0-3 matmuls cover the LN-chain gap while KV half 1 lands
        av0 = pav.tile([65, 1024], F32, tag="av", name="av")
        attn_kts(0, 0, av0, range(0, 4))
        av1 = pav.tile([65, 1024], F32, tag="av", name="av")
        attn_kts(1, 0, av1, range(0, 4))
        _emit._kv1(1)
        attn_kts(0, 0, av0, range(4, 8))
        attn_evac(0, 0, av0)
        attn_kts(1, 0, av1, range(4, 8))
        attn_evac(1, 0, av1)
        for hp in (2, 3):
            av = pav.tile([65, 1024], F32, tag="av", name="av")
            attn_kts(hp, 0, av, range(8))
            attn_evac(hp, 0, av)

        for qc in range(1, 4):           # remaining 512-wide query chunks
            for hp in range(4):
                av = pav.tile([65, 1024], F32, tag="av", name="av")
                attn_kts(hp, qc, av, range(8))
                attn_evac(hp, qc, av)
                # interleave the previous half's projection into this chunk
                if qc == 2:
                    proj_chunk(0, hp)
                elif qc == 3:
                    proj_chunk(1, hp, nns=(0,))
        for ot in range(4):
            proj_chunk(1, ot, nns=(1,))

    if dbg is not None:
        for i in range(4):
            nc.sync.dma_start(out=dbg[f"dbg_xraw{i}"], in_=x_raw[i][:].bitcast(F32))
            nc.sync.dma_start(out=dbg[f"dbg_qT{i}"], in_=qT[i][:])
            nc.sync.dma_start(out=dbg[f"dbg_kT{i}"][:, 0:512], in_=kT0[i][:])
            nc.sync.dma_start(out=dbg[f"dbg_kT{i}"][:, 512:1024], in_=kT1[i][:])
        for i in range(8):
            nc.sync.dma_start(out=dbg[f"dbg_v{i}"], in_=v_sb[i][:])


_CACHE = {}


def _build(debug=False):
    key = ("nc", debug)
    if key in _CACHE:
        return _CACHE[key]
    nc = bacc.Bacc("TRN2", target_bir_lowering=False, debug=False, num_devices=8)
    io = (
        nc.dram_tensor("xq", [C, NQ], BF16, kind="ExternalInput").ap(),
        nc.dram_tensor("xo", [C, NQ], BF16, kind="ExternalInput").ap(),
        nc.dram_tensor("w2", [4 * C, C], BF16, kind="ExternalInput").ap(),
        nc.dram_tensor("wq", [C, C], BF16, kind="ExternalInput").ap(),
        nc.dram_tensor("wkg", [C, C], BF16, kind="ExternalInput").ap(),
        nc.dram_tensor("wkg2", [2, C], BF16, kind="ExternalInput").ap(),
        nc.dram_tensor("wvg", [C, C], BF16, kind="ExternalInput").ap(),
        nc.dram_tensor("wvg2", [2, C], BF16, kind="ExternalInput").ap(),
        nc.dram_tensor("wp", [C, C], BF16, kind="ExternalInput").ap(),
        nc.dram_tensor("bsr_t", [128, 4], F32, kind="ExternalInput").ap(),
        nc.dram_tensor("bp_t", [128, 4], F32, kind="ExternalInput").ap(),
        nc.dram_tensor("ones_row", [1, NKV], BF16, kind="ExternalInput").ap(),
        nc.dram_tensor("ones_col", [128, 8], BF16, kind="ExternalInput").ap(),
        nc.dram_tensor("ones_c1", [128, 1], F32R, kind="ExternalInput").ap(),
        nc.dram_tensor("yt", [C, NQ], F32, kind="ExternalOutput").ap(),
    )
    dbg = None
    if debug:
        dbg = {}
        for i in range(4):
            dbg[f"dbg_xraw{i}"] = nc.dram_tensor(
                f"dbg_xraw{i}", [128, NKV], F32, kind="ExternalOutput").ap()
            dbg[f"dbg_qT{i}"] = nc.dram_tensor(
                f"dbg_qT{i}", [128, NQ], BF16, kind="ExternalOutput").ap()
            dbg[f"dbg_kT{i}"] = nc.dram_tensor(
                f"dbg_kT{i}", [128, NKV], BF16, kind="ExternalOutput").ap()
        for i in range(8):
            dbg[f"dbg_v{i}"] = nc.dram_tensor(
                f"dbg_v{i}", [128, NHEAD, DH + 1], BF16, kind="ExternalOutput").ap()
    with tile.TileContext(nc) as tc, ExitStack() as ctx:
        _emit(nc, tc, ctx, io, dbg)
    nc.compile()
    _CACHE[key] = nc
    return nc


def _prep_inputs(x, Wq, Wkv, Wsr, bsr, ln_g, ln_b, Wp, bp):
    x = np.asarray(x, np.float32)
    Wq = np.asarray(Wq, np.float32)
    Wkv = np.asarray(Wkv, np.float32)
    Wsr = np.asarray(Wsr, np.float32)
    bsr = np.asarray(bsr, np.float32)
    ln_g = np.asarray(ln_g, np.float32)
    ln_b = np.asarray(ln_b, np.float32)
    Wp = np.asarray(Wp, np.float32)
    bp = np.asarray(bp, np.float32)

    w2 = np.ascontiguousarray(Wsr.transpose(2, 3, 1, 0).reshape(4 * C, C).astype(BF))
    wq = np.ascontiguousarray(Wq.T.astype(BF))
    Wk, Wv = Wkv[:C], Wkv[C:]

    def ext(W):
        main = np.ascontiguousarray((W * ln_g[None, :]).T.astype(BF))   # [c, o]
        rows = np.stack([W @ ln_g, W @ ln_b]).astype(BF)                # [2, o]
        return main, np.ascontiguousarray(rows)

    wkg, wkg2 = ext(Wk)
    wvg, wvg2 = ext(Wv)
    wp = np.ascontiguousarray(Wp.T.astype(BF))
    bsr_t = np.ascontiguousarray(bsr.reshape(4, 128).T)
    bp_t = np.ascontiguousarray(bp.reshape(4, 128).T)

    shared = dict(w2=w2, wq=wq, wkg=wkg, wkg2=wkg2, wvg=wvg, wvg2=wvg2,
                  wp=wp, bsr_t=bsr_t, bp_t=bp_t,
                  ones_row=np.ones((1, NKV), BF),
                  ones_col=np.ones((128, 8), BF),
                  ones_c1=np.ones((128, 1), np.float32))
    in_maps = []
    for core in range(8):
        b, half = core // 2, core % 2
        xT = x[b].T.astype(BF)                # [C, NTOK]
        m = dict(shared)
        m["xq"] = np.ascontiguousarray(xT[:, half * NQ:(half + 1) * NQ])
        m["xo"] = np.ascontiguousarray(xT[:, (1 - half) * NQ:(2 - half) * NQ])
        in_maps.append(m)
    return in_maps


def kernel(x, H, W, Wq, Wkv, Wsr, bsr, ln_g, ln_b, Wp, bp, _trace=False, _debug=False):
    nc = _build(debug=_debug)
    in_maps = _prep_inputs(x, Wq, Wkv, Wsr, bsr, ln_g, ln_b, Wp, bp)
    res = run_bass_kernel_spmd(nc, in_maps, list(range(8)), trace=_trace)
    y = np.empty((B, NTOK, C), np.float32)
    for core in range(8):
        b, half = core // 2, core % 2
        y[b, half * NQ:(half + 1) * NQ, :] = res.results[core]["yt"].T
    kernel._last_result = res
    if _debug:
        kernel._debug_out = {k: np.asarray(v) for k, v in res.results[0].items()
                             if k.startswith("dbg_")}
    return y


# revision 23
# speedup vs baseline: 1.0626x; 1.0626x over previous
"""Trainium2 Bass kernel for AttentionSR (spatial-reduction attention).

Reference computation (per batch b):
  q = x @ Wq.T                                   [4096, 512] -> heads [8, 4096, 64]
  x_ = conv2x2_stride2(x as NCHW image, Wsr) + bsr   -> [1024, 512]
  x_ = layernorm(x_, g, b)
  k, v = split(x_ @ Wkv.T)                       [8, 1024, 64] each
  out = softmax(q k^T / 8) v                     -> [4096, 512]
  y = out @ Wp.T + bp

Sharding (8 cores): core = 2*batch + query_half. Each core owns one batch's
conv/LN/KV (duplicated across the pair) and 2048 of its 4096 query rows.
No collectives.

On-device layout is channel-major throughout (host passes x transposed), so
the kernel needs no PE transposes:
  conv/q/k projections keep channels on partitions; v is produced token-major
  by swapping the stationary matmul operand; attention scores are computed
  transposed [keys, q]; the softmax denominator comes from a ones column
  appended to v in the attn@v stationary operand; the final division is an
  approx-reciprocal + gpsimd partition-broadcast + multiply on [64, q] tiles.
Matmul paths run in bf16 (weight loads overlap via FWL, N=1024 moving
operands); LN statistics run in float32r for accuracy; psums are fp32.
LayerNorm is folded into the KV projection: x_scaled = x_raw * rstd plus two
extension rows (s2 = -mu*rstd, ones) against host-extended weights
[ (W*g).T ; W@g ; W@b ].
Attention processes head pairs: the two K=64 score matmuls row-pack into
array rows 0-63 / 64-127 (concurrent), and both heads' attn@v accumulate
into one [65, 2048] psum.
"""

import numpy as np
import ml_dtypes
from contextlib import ExitStack

import concourse.bass as bass
import concourse.bacc as bacc
import concourse.tile as tile
from concourse import mybir
from concourse.bass_utils import run_bass_kernel_spmd

BF = ml_dtypes.bfloat16
F32 = mybir.dt.float32
F32R = mybir.dt.float32r
BF16 = mybir.dt.bfloat16
AF = mybir.ActivationFunctionType
ALU = mybir.AluOpType

C = 512          # model dim
NHEAD = 8
DH = 64          # head dim
HS = WS = 64     # image height/width
NTOK = HS * WS   # 4096 tokens per batch
NQ = 2048        # query rows per core
NKV = 1024       # reduced tokens (keys)
B = 4
SCALE = DH ** -0.5
EPS = 1e-5


def _emit(nc, tc, ctx, io, dbg=None):
    (xq, xo, w2, wq, wkg, wkg2, wvg, wvg2, wp, bsr_t, bp_t,
     ones_row, ones_col, ones_c1, yt) = io

    pp = ctx.enter_context(tc.tile_pool(name="pp", bufs=2, space="PSUM"))
    pav = ctx.enter_context(tc.tile_pool(name="pav", bufs=2, space="PSUM"))
    persist = ctx.enter_context(tc.tile_pool(name="persist", bufs=1))
    small = ctx.enter_context(tc.tile_pool(name="small", bufs=1))

    # ---- persistent sbuf tensors ----
    qT = [persist.tile([128, NQ], BF16, tag=f"qT{i}", name=f"qT{i}") for i in range(4)]
    kT0 = [persist.tile([128, 512], BF16, tag=f"kT0{i}", name=f"kT0{i}") for i in range(4)]
    kT1 = [persist.tile([128, 512], BF16, tag=f"kT1{i}", name=f"kT1{i}") for i in range(4)]
    v_sb = [persist.tile([128, NHEAD, DH + 1], BF16, tag=f"v{i}", name=f"v{i}")
            for i in range(8)]
    vout = [[persist.tile([128, 1024], BF16, tag=f"vout{i}_{h}", name=f"vout{i}_{h}")
             for h in range(2)] for i in range(4)]
    x_raw = [persist.tile([128, NKV], F32R, tag=f"xraw{i}", name=f"xraw{i}")
             for i in range(4)]
    wp_sb = persist.tile([128, 4, 512], BF16, tag="wp")

    bsr_sb = small.tile([128, 4], F32)
    nc.sync.dma_start(out=bsr_sb[:], in_=bsr_t)
    bp_sb = small.tile([128, 4], F32)
    nc.sync.dma_start(out=bp_sb[:], in_=bp_t)
    ones_c = small.tile([128, 1], F32R)
    nc.sync.dma_start(out=ones_c[:], in_=ones_c1)
    # LN row tensors ([1, N] tiles, base partition 0; values overwritten in place)
    sum_row = small.tile([1, NKV], F32)    # sum -> m
    sq_row = small.tile([1, NKV], F32)     # sumsq -> var -> std -> rstd
    msq_row = small.tile([1, NKV], F32)    # m^2 / newton scratch
    y_row = small.tile([1, NKV], F32)      # rsqrt newton iterate
    rstd_bc = small.tile([128, NKV], F32)
    xs_ext2 = small.tile([2, NKV], BF16)   # row0 = -mu*rstd, row1 = ones (DMA)
    nc.sync.dma_start(out=xs_ext2[1:2, :], in_=ones_row)

    # ========== Phase A+B: per-half pipeline: conv -> LN -> KV; then q ==========
    xs_ln = [persist.tile([128, NKV], BF16, tag=f"xsln{i}", name=f"xsln{i}")
             for i in range(4)]
    with tc.tile_pool(name="pA", bufs=1) as pA, \
         tc.tile_pool(name="pB", bufs=4) as pB, \
         tc.tile_pool(name="xstream", bufs=2) as pX:
        xh0 = pX.tile([128, 4, NQ], BF16, tag="xh", name="xh")
        nc.sync.dma_start(out=xh0[:], in_=xq.rearrange("(ct p) t -> p ct t", p=128))
        w2_sb = [pA.tile([128, 4, 512], BF16, tag=f"w2_{i}", name=f"w2_{i}")
                 for i in range(4)]
        w2v = w2.rearrange("(dd ct p) o -> dd p ct o", dd=4, p=128)
        for i in range(4):
            nc.sync.dma_start(out=w2_sb[i][:], in_=w2v[i])
        wq_sb = pA.tile([128, 4, 512], BF16)
        nc.sync.dma_start(out=wq_sb[:], in_=wq.rearrange("(ct p) o -> p ct o", p=128))
        wkg_sb = persist.tile([128, 4, 512], BF16, tag="wkg", name="wkg_sb")
        nc.sync.dma_start(out=wkg_sb[:], in_=wkg.rearrange("(ct p) o -> p ct o", p=128))
        wkg2_sb = persist.tile([2, 512], BF16, tag="wkg2", name="wkg2_sb")
        nc.sync.dma_start(out=wkg2_sb[:], in_=wkg2)
        wvg_sb = persist.tile([128, 4, 512], BF16, tag="wvg", name="wvg_sb")
        nc.sync.dma_start(out=wvg_sb[:], in_=wvg.rearrange("(ct p) o -> p ct o", p=128))
        wvg2_sb = persist.tile([2, 512], BF16, tag="wvg2", name="wvg2_sb")
        nc.sync.dma_start(out=wvg2_sb[:], in_=wvg2)

        inv_c = 1.0 / C
        xh_q = None

        def conv_half(half, xh):
            hsl = slice(half * 512, (half + 1) * 512)
            for ot in range(4):
                ps = pp.tile([128, 1024], F32, tag="ps", name="ps_conv")
                psv = ps[:, 0:512].rearrange("p (a b) -> p a b", a=16)
                for kk in range(16):
                    di, dj, ct = kk // 8, (kk // 4) % 2, kk % 4
                    rhs = bass.AP(
                        tensor=xh[:].tensor,
                        offset=xh[:].offset + ct * NQ + di * WS + dj,
                        ap=[xh[:].ap[0], [2 * WS, 16], [2, 32]],
                    )
                    nc.tensor.matmul(
                        psv, lhsT=w2_sb[kk // 4][:, kk % 4, ot * 128:(ot + 1) * 128],
                        rhs=rhs, start=(kk == 0), stop=(kk == 15),
                    )
                nc.vector.tensor_scalar_add(
                    x_raw[ot][:, hsl], ps[:, 0:512], bsr_sb[:, ot:ot + 1],
                )

        def stats_half(half):
            hsl = slice(half * 512, (half + 1) * 512)
            xsq = []
            for ct in range(4):
                t = pB.tile([128, 512], F32R, tag="xsq", name="xsq")
                nc.vector.tensor_mul(t[:], x_raw[ct][:, hsl].bitcast(F32),
                                     x_raw[ct][:, hsl].bitcast(F32))
                xsq.append(t)
            ps = pp.tile([128, 1024], F32, tag="ps", name="ps_sum")
            for ct in range(4):
                nc.tensor.matmul(ps[0:1, 0:512], lhsT=ones_c[:], rhs=x_raw[ct][:, hsl],
                                 start=(ct == 0), stop=(ct == 3))
            for ct in range(4):
                nc.tensor.matmul(ps[0:1, 512:1024], lhsT=ones_c[:], rhs=xsq[ct][:],
                                 start=(ct == 0), stop=(ct == 3))
            nc.vector.tensor_copy(sum_row[0:1, hsl], ps[0:1, 0:512])
            nc.vector.tensor_copy(sq_row[0:1, hsl], ps[0:1, 512:1024])

        def ln_rows_half(half):
            hsl = slice(half * 512, (half + 1) * 512)
            sm, sq, mq = sum_row[0:1, hsl], sq_row[0:1, hsl], msq_row[0:1, hsl]
            yy = y_row[0:1, hsl]
            nc.vector.tensor_scalar_mul(sm, sm, inv_c)              # m
            nc.vector.tensor_mul(mq, sm, sm)                        # m^2
            nc.vector.scalar_tensor_tensor(sq, sq, inv_c, mq,
                                           op0=ALU.mult, op1=ALU.subtract)   # var
            nc.vector.tensor_scalar_add(sq, sq, EPS)                # var + eps
            # rstd = rsqrt(var+eps): seed 1/v, 3 Newton steps
            nc.vector.reciprocal_approx_fast(out=yy, in_=sq)
            for _ in range(3):
                nc.vector.tensor_mul(mq, yy, yy)
                nc.vector.scalar_tensor_tensor(mq, sq, -0.5, mq,
                                               op0=ALU.mult, op1=ALU.mult)
                nc.vector.scalar_tensor_tensor(yy, mq, 1.5, yy,
                                               op0=ALU.add, op1=ALU.mult)
            nc.vector.scalar_tensor_tensor(xs_ext2[0:1, hsl], sm, -1.0, yy,
                                           op0=ALU.mult, op1=ALU.mult)       # s2
            nc.gpsimd.partition_broadcast(rstd_bc[:, hsl], yy)
            for ct in range(4):
                nc.vector.tensor_mul(xs_ln[ct][:, hsl],
                                     x_raw[ct][:, hsl].bitcast(F32),
                                     rstd_bc[:, hsl])

        def kv_half(half):
            hsl = slice(half * 512, (half + 1) * 512)
            kTh = kT0 if half == 0 else kT1
            for ot in range(4):
                ps = pp.tile([128, 1024], F32, tag="ps", name="ps_k")
                for ct in range(4):
                    nc.tensor.matmul(ps[:, 0:512],
                                     lhsT=wkg_sb[:, ct, ot * 128:(ot + 1) * 128],
                                     rhs=xs_ln[ct][:, hsl],
                                     start=(ct == 0), stop=False)
                nc.tensor.matmul(ps[:, 0:512],
                                 lhsT=wkg2_sb[:, ot * 128:(ot + 1) * 128],
                                 rhs=xs_ext2[:, hsl], start=False, stop=True)
                nc.vector.tensor_copy(kTh[ot][:], ps[:, 0:512])
            for tt in range(half * 4, half * 4 + 4):
                sl = slice(tt * 128, (tt + 1) * 128)
                ps = pp.tile([128, 1024], F32, tag="ps", name="ps_v")
                for ct in range(4):
                    nc.tensor.matmul(ps[:, 0:512], lhsT=xs_ln[ct][:, sl],
                                     rhs=wvg_sb[:, ct, :], start=(ct == 0), stop=False)
                nc.tensor.matmul(ps[:, 0:512], lhsT=xs_ext2[:, sl], rhs=wvg2_sb[:],
                                 start=False, stop=True)
                nc.vector.tensor_copy(
                    v_sb[tt][:, :, 0:DH],
                    ps[:, 0:512].rearrange("p (h d) -> p h d", h=NHEAD),
                )

        xh1 = pX.tile([128, 4, NQ], BF16, tag="xh", name="xh1")
        nc.sync.dma_start(out=xh1[:], in_=xo.rearrange("(ct p) t -> p ct t", p=128))
        for i in range(8):
            nc.sync.dma_start(out=v_sb[i][:, :, DH:DH + 1], in_=ones_col)
        conv_half(0, xh0)
        stats_half(0)
        ln_rows_half(0)          # DVE chain overlaps the q/conv1 matmuls below
        for ot in range(4):      # q projection early (unblocks attention sooner)
            for qc in range(2):
                ps = pp.tile([128, 1024], F32, tag="ps", name="ps_q")
                for ct in range(4):
                    for nn in range(2):
                        nc.tensor.matmul(
                            ps[:, nn * 512:(nn + 1) * 512],
                            lhsT=wq_sb[:, ct, ot * 128:(ot + 1) * 128],
                            rhs=xh0[:, ct, qc * 1024 + nn * 512:
                                    qc * 1024 + nn * 512 + 512],
                            start=(ct == 0), stop=(ct == 3),
                        )
                nc.vector.tensor_copy(qT[ot][:, qc * 1024:(qc + 1) * 1024], ps[:])
        conv_half(1, xh1)
        kv_half(0)
        stats_half(1)
        ln_rows_half(1)
        _emit._kv1 = kv_half

    nc.sync.dma_start(out=wp_sb[:], in_=wp.rearrange("(ct p) o -> p ct o", p=128))
    # ================= Phase C: attention (head pairs, 512-col q chunks) ======
    with tc.tile_pool(name="pexp", bufs=8) as pexp, \
         tc.tile_pool(name="psig", bufs=2) as psig, \
         tc.tile_pool(name="py", bufs=3) as py:

        def proj_chunk(qh, ot):
            ps = pp.tile([128, 1024], F32, tag="ps", name="ps_proj")
            for ct in range(4):
                for nn in range(2):
                    nc.tensor.matmul(
                        ps[:, nn * 512:(nn + 1) * 512],
                        lhsT=wp_sb[:, ct, ot * 128:(ot + 1) * 128],
                        rhs=vout[ct][qh][:, nn * 512:(nn + 1) * 512],
                        start=(ct == 0), stop=(ct == 3))
            yt_t = py.tile([128, 1024], F32, tag="y", name="yt_t")
            nc.vector.tensor_scalar_add(yt_t[:], ps[:], bp_sb[:, ot:ot + 1])
            nc.sync.dma_start(
                out=yt[ot * 128:(ot + 1) * 128, qh * 1024:(qh + 1) * 1024],
                in_=yt_t[:])

        def attn_kts(hp, qc, av, kts):
            qsl = slice(qc * 512, (qc + 1) * 512)
            for kt in kts:
                kTh = kT0 if kt < 4 else kT1
                ksl = slice((kt % 4) * 128, (kt % 4) * 128 + 128)
                sc = pp.tile([128, 1024], F32, tag="ps", name="sc")
                # both heads' score matmuls adjacent -> concurrent row groups
                for sub in range(2):
                    rr = sub * 64
                    nc.tensor.matmul(
                        sc[:, sub * 512:(sub + 1) * 512],
                        lhsT=kTh[hp][rr:rr + 64, ksl],
                        rhs=qT[hp][rr:rr + 64, qsl],
                        start=True, stop=True,
                    )
                ex = pexp.tile([128, 1024], BF16, tag="exp", name="ex")
                nc.scalar.activation(ex[:], sc[:], AF.Exp, scale=SCALE)
                for sub in range(2):
                    nc.tensor.matmul(
                        av[:, sub * 512:(sub + 1) * 512],
                        lhsT=v_sb[kt][:, 2 * hp + sub, :],
                        rhs=ex[:, sub * 512:(sub + 1) * 512],
                        start=(kt == 0), stop=(kt == 7),
                    )

        def attn_evac(hp, qc, av):
            qh, qr = qc // 2, (qc % 2) * 512
            sig = psig.tile([1, 1024], F32, tag="sig", name="sig")
            nc.vector.tensor_copy(sig[:], av[64:65, :])
            rbc = psig.tile([64, 1024], F32, tag="rbc", name="rbc")
            nc.vector.reciprocal_approx_fast(out=rbc[0:1, :], in_=sig[:])
            nc.gpsimd.partition_broadcast(rbc[:], rbc[0:1, :])
            nc.vector.tensor_mul(vout[hp][qh][0:64, qr:qr + 512],
                                 av[0:64, 0:512], rbc[:, 0:512])
            nc.vector.tensor_mul(vout[hp][qh][64:128, qr:qr + 512],
                                 av[0:64, 512:1024], rbc[:, 512:1024])

        # qc=0: first two passes split around the deferred second-half KV so
        # their kt0-3 matmuls cover the LN-chain gap while KV half 1 lands
        av0 = pav.tile([65, 1024], F32, tag="av", name="av")
        attn_kts(0, 0, av0, range(0, 4))
        av1 = pav.tile([65, 1024], F32, tag="av", name="av")
        attn_kts(1, 0, av1, range(0, 4))
        _emit._kv1(1)
        attn_kts(0, 0, av0, range(4, 8))
        attn_evac(0, 0, av0)
        attn_kts(1, 0, av1, range(4, 8))
        attn_evac(1, 0, av1)
        for hp in (2, 3):
            av = pav.tile([65, 1024], F32, tag="av", name="av")
            attn_kts(hp, 0, av, range(8))
            attn_evac(hp, 0, av)

        for qc in range(1, 4):           # remaining 512-wide query chunks
            for hp in range(4):
                av = pav.tile([65, 1024], F32, tag="av", name="av")
                attn_kts(hp, qc, av, range(8))
                attn_evac(hp, qc, av)
                # interleave the previous half's projection into this chunk
                if qc == 2:
                    proj_chunk(0, hp)
            if qc == 3:
                for ot in range(4):
                    proj_chunk(1, ot)

    if dbg is not None:
        for i in range(4):
            nc.sync.dma_start(out=dbg[f"dbg_xraw{i}"], in_=x_raw[i][:].bitcast(F32))
            nc.sync.dma_start(out=dbg[f"dbg_qT{i}"], in_=qT[i][:])
            nc.sync.dma_start(out=dbg[f"dbg_kT{i}"][:, 0:512], in_=kT0[i][:])
            nc.sync.dma_start(out=dbg[f"dbg_kT{i}"][:, 512:1024], in_=kT1[i][:])
        for i in range(8):
            nc.sync.dma_start(out=dbg[f"dbg_v{i}"], in_=v_sb[i][:])


_CACHE = {}


def _build(debug=False):
    key = ("nc", debug)
    if key in _CACHE:
        return _CACHE[key]
    nc = bacc.Bacc("TRN2", target_bir_lowering=False, debug=False, num_devices=8)
    io = (
        nc.dram_tensor("xq", [C, NQ], BF16, kind="ExternalInput").ap(),
        nc.dram_tensor("xo", [C, NQ], BF16, kind="ExternalInput").ap(),
        nc.dram_tensor("w2", [4 * C, C], BF16, kind="ExternalInput").ap(),
        nc.dram_tensor("wq", [C, C], BF16, kind="ExternalInput").ap(),
        nc.dram_tensor("wkg", [C, C], BF16, kind="ExternalInput").ap(),
        nc.dram_tensor("wkg2", [2, C], BF16, kind="ExternalInput").ap(),
        nc.dram_tensor("wvg", [C, C], BF16, kind="ExternalInput").ap(),
        nc.dram_tensor("wvg2", [2, C], BF16, kind="ExternalInput").ap(),
        nc.dram_tensor("wp", [C, C], BF16, kind="ExternalInput").ap(),
        nc.dram_tensor("bsr_t", [128, 4], F32, kind="ExternalInput").ap(),
        nc.dram_tensor("bp_t", [128, 4], F32, kind="ExternalInput").ap(),
        nc.dram_tensor("ones_row", [1, NKV], BF16, kind="ExternalInput").ap(),
        nc.dram_tensor("ones_col", [128, 8], BF16, kind="ExternalInput").ap(),
        nc.dram_tensor("ones_c1", [128, 1], F32R, kind="ExternalInput").ap(),
        nc.dram_tensor("yt", [C, NQ], F32, kind="ExternalOutput").ap(),
    )
    dbg = None
    if debug:
        dbg = {}
        for i in range(4):
            dbg[f"dbg_xraw{i}"] = nc.dram_tensor(
                f"dbg_xraw{i}", [128, NKV], F32, kind="ExternalOutput").ap()
            dbg[f"dbg_qT{i}"] = nc.dram_tensor(
                f"dbg_qT{i}", [128, NQ], BF16, kind="ExternalOutput").ap()
            dbg[f"dbg_kT{i}"] = nc.dram_tensor(
                f"dbg_kT{i}", [128, NKV], BF16, kind="ExternalOutput").ap()
        for i in range(8):
            dbg[f"dbg_v{i}"] = nc.dram_tensor(
                f"dbg_v{i}", [128, NHEAD, DH + 1], BF16, kind="ExternalOutput").ap()
    with tile.TileContext(nc) as tc, ExitStack() as ctx:
        _emit(nc, tc, ctx, io, dbg)
    nc.compile()
    _CACHE[key] = nc
    return nc


def _prep_inputs(x, Wq, Wkv, Wsr, bsr, ln_g, ln_b, Wp, bp):
    x = np.asarray(x, np.float32)
    Wq = np.asarray(Wq, np.float32)
    Wkv = np.asarray(Wkv, np.float32)
    Wsr = np.asarray(Wsr, np.float32)
    bsr = np.asarray(bsr, np.float32)
    ln_g = np.asarray(ln_g, np.float32)
    ln_b = np.asarray(ln_b, np.float32)
    Wp = np.asarray(Wp, np.float32)
    bp = np.asarray(bp, np.float32)

    w2 = np.ascontiguousarray(Wsr.transpose(2, 3, 1, 0).reshape(4 * C, C).astype(BF))
    wq = np.ascontiguousarray(Wq.T.astype(BF))
    Wk, Wv = Wkv[:C], Wkv[C:]

    def ext(W):
        main = np.ascontiguousarray((W * ln_g[None, :]).T.astype(BF))   # [c, o]
        rows = np.stack([W @ ln_g, W @ ln_b]).astype(BF)                # [2, o]
        return main, np.ascontiguousarray(rows)

    wkg, wkg2 = ext(Wk)
    wvg, wvg2 = ext(Wv)
    wp = np.ascontiguousarray(Wp.T.astype(BF))
    bsr_t = np.ascontiguousarray(bsr.reshape(4, 128).T)
    bp_t = np.ascontiguousarray(bp.reshape(4, 128).T)

    shared = dict(w2=w2, wq=wq, wkg=wkg, wkg2=wkg2, wvg=wvg, wvg2=wvg2,
                  wp=wp, bsr_t=bsr_t, bp_t=bp_t,
                  ones_row=np.ones((1, NKV), BF),
                  ones_col=np.ones((128, 8), BF),
                  ones_c1=np.ones((128, 1), np.float32))
    in_maps = []
    for core in range(8):
        b, half = core // 2, core % 2
        xT = x[b].T.astype(BF)                # [C, NTOK]
        m = dict(shared)
        m["xq"] = np.ascontiguousarray(xT[:, half * NQ:(half + 1) * NQ])
        m["xo"] = np.ascontiguousarray(xT[:, (1 - half) * NQ:(2 - half) * NQ])
        in_maps.append(m)
    return in_maps


def kernel(x, H, W, Wq, Wkv, Wsr, bsr, ln_g, ln_b, Wp, bp, _trace=False, _debug=False):
    nc = _build(debug=_debug)
    in_maps = _prep_inputs(x, Wq, Wkv, Wsr, bsr, ln_g, ln_b, Wp, bp)
    res = run_bass_kernel_spmd(nc, in_maps, list(range(8)), trace=_trace)
    y = np.empty((B, NTOK, C), np.float32)
    for core in range(8):
        b, half = core // 2, core % 2
        y[b, half * NQ:(half + 1) * NQ, :] = res.results[core]["yt"].T
    kernel._last_result = res
    if _debug:
        kernel._debug_out = {k: np.asarray(v) for k, v in res.results[0].items()
                             if k.startswith("dbg_")}
    return y
